# revision 1
# baseline (speedup 1.0000x reference)
"""4-layer GAT + MLP head on Trainium2, 8-core SPMD (dst-sharded graph parallel).

Strategy (v2):
  * Nodes sharded across 8 cores (6250/core); edges sharded by destination.
  * Layer 0: x is tiny (50000x10), so every core receives the FULL x (rolled
    so its own shard comes first) and computes the full 50000-row
    [h | alpha_s] table locally -- no AllGather and no barrier.  The roll
    makes the core's own rows land at table rows [0:6250], so the local
    alpha_d table write needs no per-core branching.
  * Layers 1-3: per layer each core computes h = x @ W_aug (bf16) for its
    node shard, writes node-major chunks to HBM; the table AllGather is
    split at the int16 gather-half boundary into two Shared tensors
    ([32, 17] blocks -> 32768/17232 rows): the lo chunk is gathered as soon
    as its rows are written (overlapping the tail of the previous layer's
    aggregation), only the hi chunk sits at the layer boundary.
  * Aggregation (software-pipelined 2-stage: gathers+DVE prep for block b
    run while PE matmuls+finish for block b-1 drain): dma_gather pulls
    h[src] rows, alpha = leaky_relu(a_s[src]+a_d[dst]), e = exp(alpha)
    written into the row's alpha columns, Y = e * h[src], and a one-hot
    selection matrix S (built by is_equal in bf16; features stored
    head-interleaved so the per-head broadcasts keep the packed innermost
    dim and the DVE 2x mode) turns the segment-sum into PSUM-accumulated
    matmuls.  For layers 0/3 the softmax denominator rides in the same
    matmul (rhs width fo+h); layers 1/2 keep a separate tiny matmul
    (fo=512 fills a PSUM bank).  An optional two-phase lo/hi spill variant
    exists behind ablate=("spill",) but regressed real hardware.
  * Softmax max-subtraction is skipped (alphas are O(1); exp is safe and
    the result is mathematically identical).
  * ELU is computed as relu(y) + min(exp(y),1) - 1 with the "-1" folded into
    the next layer's matmul via a column-sum correction row.
"""

import math
from contextlib import ExitStack

import numpy as np

P = 128
NCORES = 8


class Cfg:
    def __init__(self, N=50000, F_IN=10, HS=64, split=32768, neg=0.2, bf16=True):
        self.N, self.F_IN, self.HS, self.split, self.neg = N, F_IN, HS, split, neg
        self.bf16 = bf16
        assert N % NCORES == 0
        self.NPC = N // NCORES
        self.NBLK = math.ceil(self.NPC / P)
        self.ADPAD = self.NPC              # pad row in ad table (= -1e30)
        self.ADROWS = self.NPC + 8
        # layers: (f_in, f_out_total, heads)
        self.layers = [
            (F_IN, 4 * HS, 4),
            (4 * HS, 8 * HS, 4),
            (8 * HS, 8 * HS, 4),
            (8 * HS, HS, 1),
        ]
        # gather-table row widths (elems, 256B-aligned)
        al = 128 if bf16 else 64
        self.RS = [((fo + h + al - 1) // al) * al for (_, fo, h) in self.layers]
        self.ADWE = 128 if bf16 else 64   # ad-table row elems (256B)
        # AllGather row-chunks for layers 1-3: split exactly at the int16
        # gather-half boundary (8*4096 = 32768) so each chunk is its own
        # Shared tensor (single collective writer) and is also the gather
        # table for that half.
        self.agch_blocks = [32, 17]
        assert sum(self.agch_blocks) == self.NBLK
        bounds = np.concatenate([[0], np.cumsum(self.agch_blocks)]) * P
        bounds = np.minimum(bounds, self.NPC)
        self.ag_bounds = bounds.astype(np.int64)          # local row bounds
        self.blk2chunk = np.repeat(np.arange(len(self.agch_blocks)),
                                   self.agch_blocks)
        self.chunk_last_blk = (np.cumsum(self.agch_blocks) - 1).tolist()

    def chunk_map(self, n):
        """Global node id -> hfull row for the chunked-AG layout."""
        c, r = n // self.NPC, n % self.NPC
        k = np.searchsorted(self.ag_bounds, r, side="right") - 1
        rs, re = self.ag_bounds[k], self.ag_bounds[np.minimum(k + 1, len(self.ag_bounds) - 1)]
        return NCORES * rs + c * (re - rs) + (r - rs)


CFG = Cfg()


# ------------------------------------------------------------------ host prep

def _prep_one(cfg, co, blk, dlb, dloc, idx):
    """Bucket edges by (dst block, table half of idx), pad to 128 multiples
    uniform across cores.  idx = per-edge gather-table row."""
    half = (idx >= cfg.split).astype(np.int64)
    key = (co * cfg.NBLK + blk) * 2 + half
    cnt = np.bincount(key, minlength=NCORES * cfg.NBLK * 2).reshape(
        NCORES, cfg.NBLK, 2)
    nch = -(-cnt.max(axis=0) // P)                  # [NBLK, 2] chunks (maxed)
    Cb = nch.sum(axis=1)                            # [NBLK]
    chunk_base = np.concatenate([[0], np.cumsum(Cb)]).astype(np.int64)
    total_slots = int(Cb.sum()) * P

    gb = np.zeros((cfg.NBLK, 2), np.int64)          # slot base per (blk, half)
    gb[:, 0] = chunk_base[:-1] * P
    gb[:, 1] = gb[:, 0] + nch[:, 0] * P

    order = np.argsort(key, kind="stable")
    ks = key[order]
    starts = np.r_[0, np.flatnonzero(np.diff(ks)) + 1]
    run_id = np.zeros(len(ks), np.int64)
    run_id[starts[1:]] = 1
    run_id = np.cumsum(run_id)
    rank = np.arange(len(ks)) - starts[run_id]
    slot = gb[blk[order], half[order]] + rank
    co_o = co[order]

    gidx = np.zeros((NCORES, total_slots), np.int64)
    dlocf = np.full((NCORES, total_slots), -1.0, np.float32)
    adf = np.full((NCORES, total_slots), cfg.ADPAD, np.int64)
    gidx[co_o, slot] = idx[order] - half[order] * cfg.split
    dlocf[co_o, slot] = dlb[order].astype(np.float32)
    adf[co_o, slot] = dloc[order]

    def pack16(a):  # [NCORES, S] -> [NCORES, 128, S/16] int16, replicated x8
        s = a.shape[1]
        b = a.reshape(NCORES, s // 16, 16).transpose(0, 2, 1)
        return np.ascontiguousarray(np.tile(b, (1, 8, 1))).astype(np.int16)

    def pack128(a):  # [NCORES, S] -> [NCORES, 128, S/128]
        import ml_dtypes
        s = a.shape[1]
        return np.ascontiguousarray(
            a.reshape(NCORES, s // 128, P).transpose(0, 2, 1)).astype(
                ml_dtypes.bfloat16)

    return dict(
        nch=nch, chunk_base=chunk_base, total_slots=total_slots,
        gidx=pack16(gidx), adix=pack16(adf), dloc=pack128(dlocf))


def prep_edges(cfg, edge_index):
    ei = np.asarray(edge_index)
    n = cfg.N
    src = np.concatenate([ei[0], np.arange(n, dtype=np.int64)]).astype(np.int64)
    dst = np.concatenate([ei[1], np.arange(n, dtype=np.int64)]).astype(np.int64)
    co = dst // cfg.NPC
    dloc = dst - co * cfg.NPC
    blk = dloc // P
    dlb = dloc - blk * P
    # layer 0: per-core rolled table (own shard first)
    idx0 = (src - co * cfg.NPC) % n
    # layers 1-3: chunked-AG layout
    idx1 = cfg.chunk_map(src)
    return dict(ep0=_prep_one(cfg, co, blk, dlb, dloc, idx0),
                ep1=_prep_one(cfg, co, blk, dlb, dloc, idx1))


def prep_weights(cfg, inputs):
    """Augment weights with alpha columns; compute corrections and biases.

    The fo output features are stored HEAD-INTERLEAVED (column ch_i*h + head
    instead of head*ch + ch_i) so the per-head e/denominator broadcasts land
    on a middle AP dim and the innermost dim stays packed (2x DVE mode)."""
    out = {}
    names = [("W1", "as1", "ad1", "b1"), ("W2", "as2", "ad2", "b2"),
             ("W3", "as3", "ad3", "b3"), ("W4", "as4", "ad4", "b4")]
    prev_perm = None
    for li, (wn, sn, dn, bn) in enumerate(names):
        fi, fo, h = cfg.layers[li]
        ch = fo // h
        perm = np.arange(fo).reshape(h, ch).T.reshape(-1)  # new j <- old perm[j]
        W = np.asarray(inputs[wn], np.float32)
        if prev_perm is not None:  # input features are interleaved
            W = W[prev_perm, :]
        a_s = np.asarray(inputs[sn], np.float32)
        a_d = np.asarray(inputs[dn], np.float32)
        As = np.zeros((fo, h), np.float32)
        Ad = np.zeros((fo, h), np.float32)
        for hh in range(h):
            As[hh * ch:(hh + 1) * ch, hh] = a_s[hh]
            Ad[hh * ch:(hh + 1) * ch, hh] = a_d[hh]
        waug = np.concatenate([W[:, perm], W @ As, W @ Ad], axis=1)
        import ml_dtypes
        out[f"w{li}"] = waug.astype(ml_dtypes.bfloat16)
        if li > 0:  # input is elu(y)+1; subtract column sums
            out[f"cor{li}"] = np.tile(waug.sum(axis=0)[None, :], (P, 1)).astype(np.float32)
        b = np.asarray(inputs[bn], np.float32)
        out[f"bias{li}_nz"] = bool(np.any(b != 0))
        if out[f"bias{li}_nz"]:
            out[f"bias{li}"] = np.tile(b[None, perm], (P, 1)).astype(np.float32)
        prev_perm = perm
    import ml_dtypes
    wm1 = np.asarray(inputs["Wm1"], np.float32)
    out["wm1"] = wm1.astype(ml_dtypes.bfloat16)
    out["wm2"] = np.asarray(inputs["Wm2"], np.float32).astype(ml_dtypes.bfloat16)
    out["wm3"] = np.asarray(inputs["Wm3"], np.float32).astype(ml_dtypes.bfloat16)
    bm1e = np.asarray(inputs["bm1"], np.float32) - wm1.sum(axis=0)
    out["bm1"] = np.tile(bm1e[None, :], (P, 1)).astype(np.float32)
    bm2 = np.asarray(inputs["bm2"], np.float32)
    out["bm2_nz"] = bool(np.any(bm2 != 0))
    if out["bm2_nz"]:
        out["bm2"] = np.tile(bm2[None, :], (P, 1)).astype(np.float32)
    out["bm3"] = np.full((P, 1), float(np.asarray(inputs["bm3"]).reshape(-1)[0]),
                         np.float32)
    return out


# --------------------------------------------------------------- bass program

def _install_queue_sem_patch():
    """Partition Tile's 8 DMASW sem lanes across the 4 SWDGE queues (2 each)
    so a sem is only ever incremented from one queue (runtime requirement)."""
    import concourse.tile_sem_assignment as tsa
    import concourse.mybir as mybir
    from concourse import bass_isa
    from concourse.tile_scheduler import DMAInst
    if getattr(tsa, "_q_aware", False):
        return
    orig = tsa.TileClockTick._assign_tick

    def _assign_tick_q(self, inst):
        if (isinstance(inst, DMAInst)
                and inst.engine == mybir.EngineType.Pool
                and not isinstance(inst, bass_isa.UserSyncedRemoteDMADescs)):
            q = getattr(inst, "queue_num", None) or 0
            cnt = getattr(self, "_q_cnt", None)
            if cnt is None:
                cnt = self._q_cnt = [0, 0, 0, 0]
            self.next_sw_dma_idx = 2 * q + (cnt[q] & 1)
            cnt[q] += 1
        return orig(self, inst)

    tsa.TileClockTick._assign_tick = _assign_tick_q
    tsa._q_aware = True


def build_program(cfg, ep, wmeta, ablate=(), stop_after=None, repeats=1):
    import concourse.bacc as bacc
    import concourse.mybir as mybir
    import concourse.tile as tile

    dt = mybir.dt
    f32 = dt.float32
    tdt = dt.bfloat16 if cfg.bf16 else f32
    ADWE = cfg.ADWE
    NBLK, NPC = cfg.NBLK, cfg.NPC
    HS = cfg.HS
    eps = [ep["ep0"], ep["ep1"]]
    NC16 = [e["gidx"].shape[2] for e in eps]
    NC128 = [e["dloc"].shape[2] for e in eps]

    _install_queue_sem_patch()
    nc = bacc.Bacc("TRN2", target_bir_lowering=False, debug=False,
                   enable_asserts=False, num_devices=NCORES,
                   num_swdge_queues=4)
    T = {}

    def inp(name, shape, d=f32):
        T[name] = nc.dram_tensor(name, list(shape), d, kind="ExternalInput")
        return T[name]

    inp("x", [cfg.N, cfg.F_IN])
    for s in range(2):
        inp(f"gidx{s}", [P, NC16[s]], dt.int16)
        inp(f"adix{s}", [P, NC16[s]], dt.int16)
        inp(f"dloc{s}", [P, NC128[s]], tdt)
    for li, (fi, fo, h) in enumerate(cfg.layers):
        inp(f"w{li}", [fi, fo + 2 * h], tdt)
        if li > 0:
            inp(f"cor{li}", [P, fo + 2 * h])
        if wmeta[f"bias{li}_nz"]:
            inp(f"bias{li}", [P, fo])
    inp("wm1", [HS, 4 * HS], tdt)
    inp("wm2", [4 * HS, 4 * HS], tdt)
    inp("wm3", [4 * HS, 1], tdt)
    inp("bm1", [P, 4 * HS])
    if wmeta["bm2_nz"]:
        inp("bm2", [P, 4 * HS])
    inp("bm3", [P, 1])
    inp("iota", [P, P], tdt)
    inp("ident", [P, P])
    inp("negrow", [1, ADWE], tdt)
    out_t = nc.dram_tensor("out", [NPC, 1], f32, kind="ExternalOutput")

    add, mult, sub = mybir.AluOpType.add, mybir.AluOpType.mult, mybir.AluOpType.subtract
    is_eq, vmax = mybir.AluOpType.is_equal, mybir.AluOpType.max
    EXP = mybir.ActivationFunctionType.Exp
    CPY = mybir.ActivationFunctionType.Copy
    RELU = mybir.ActivationFunctionType.Relu
    SIGM = mybir.ActivationFunctionType.Sigmoid

    with tile.TileContext(nc) as tc, ExitStack() as ctx:
        const = ctx.enter_context(tc.tile_pool(name="const", bufs=1))
        dram = ctx.enter_context(tc.tile_pool(name="dram", bufs=1, space="DRAM"))
        ypool = ctx.enter_context(tc.tile_pool(name="y", bufs=3))
        apool = ctx.enter_context(tc.tile_pool(name="adg", bufs=2))
        spool = ctx.enter_context(tc.tile_pool(name="small", bufs=2))
        stpool = ctx.enter_context(tc.tile_pool(name="sel", bufs=3))
        bpool = ctx.enter_context(tc.tile_pool(name="blk", bufs=2))
        xpool = ctx.enter_context(tc.tile_pool(name="xT", bufs=3))
        psum = ctx.enter_context(tc.tile_pool(name="ps", bufs=1, space="PSUM"))

        # ---- constants into SBUF
        def load_const(name, shape, d=f32):
            t = const.tile(list(shape), d, tag=name)
            nc.sync.dma_start(t[:], T[name][tuple(slice(0, s) for s in shape)])
            return t

        iota_t = load_const("iota", [P, P], tdt)
        ident_t = load_const("ident", [P, P])
        gidx_t = [load_const(f"gidx{s}", [P, NC16[s]], dt.int16) for s in range(2)]
        adix_t = [load_const(f"adix{s}", [P, NC16[s]], dt.int16) for s in range(2)]
        dloc_t = [load_const(f"dloc{s}", [P, NC128[s]], tdt) for s in range(2)]
        negrow_t = load_const("negrow", [1, ADWE], tdt)
        w_t, cor_t, bias_t = {}, {}, {}
        for li, (fi, fo, h) in enumerate(cfg.layers):
            kt = math.ceil(fi / P)
            pd = min(fi, P)
            wt = const.tile([pd, kt, fo + 2 * h], tdt, tag=f"w{li}")
            nc.sync.dma_start(
                wt[:], T[f"w{li}"][:, :].rearrange("(k p) f -> p k f", p=pd))
            w_t[li] = wt
            if li > 0:
                cor_t[li] = load_const(f"cor{li}", [P, fo + 2 * h])
            if wmeta[f"bias{li}_nz"]:
                bias_t[li] = load_const(f"bias{li}", [P, fo])
        wm1_t = load_const("wm1", [HS, 4 * HS], tdt)
        wm2_t = const.tile([P, 2, 4 * HS], tdt, tag="wm2")
        nc.sync.dma_start(wm2_t[:], T["wm2"][:, :].rearrange("(k p) f -> p k f", p=P))
        wm3_t = const.tile([P, 2, 1], tdt, tag="wm3")
        nc.sync.dma_start(wm3_t[:], T["wm3"][:, :].rearrange("(k p) f -> p k f", p=P))
        bm1_t = load_const("bm1", [P, 4 * HS])
        bm2_t = load_const("bm2", [P, 4 * HS]) if wmeta["bm2_nz"] else None
        bm3_t = load_const("bm3", [P, 1])

        # ---- internal DRAM
        nchunk = len(cfg.agch_blocks)
        adloc = [dram.tile([cfg.ADROWS, ADWE], tdt, name=f"adloc{li}",
                           tag=f"adloc{li}")
                 for li in range(4)]
        spillT = dram.tile([NBLK * P, 520], tdt, name="spill", tag="spill")
        hfull0 = [None]          # per-rep [N, RS0] plain table
        hfull = [None] * 4       # per-rep [lo, hi] Shared tables, layers 1-3
        hlock = [None] * 4       # per-rep per-chunk local tables

        def blocks():
            for b in range(NBLK):
                yield b, min(P, NPC - b * P)

        # ---------------- layer-0 full-table sweep (no AllGather)
        # Two-stage software pipeline: iteration t emits [dma x_t, transpose_t]
        # then [copy/matmuls/stores for t-1], so PE streams tile t's transpose
        # while DVE finishes tile t-1.
        def sweep0():
            fi, fo, h = cfg.layers[0]
            tiles = [(cc, b, pp) for cc in range(NCORES) for b, pp in blocks()]

            def front(cc, b, pp):
                base = cc * NPC + b * P
                xb = bpool.tile([P, fi], f32, tag="xb")
                nc.sync.dma_start(xb[:pp, :], T["x"][base:base + pp, :])
                ptr = psum.tile([P, P], f32, tag="ptr", bufs=2)
                nc.tensor.transpose(ptr[:fi, :pp], xb[:pp, :fi],
                                    ident_t[:pp, :pp])
                return ptr

            def back(cc, b, pp, ptr):
                base = cc * NPC + b * P
                xts = xpool.tile([P, P], tdt, tag="xts")
                nc.vector.tensor_copy(xts[:fi, :pp], ptr[:fi, :pp])
                psh = psum.tile([P, fo + 2 * h], f32, tag="psh", bufs=2)
                nc.tensor.matmul(psh[:pp, :], lhsT=xts[:fi, :pp],
                                 rhs=w_t[0][:fi, 0, :],
                                 start=True, stop=True)
                hrow = bpool.tile([P, cfg.RS[0]], tdt, tag="hrow")
                nc.vector.tensor_copy(hrow[:pp, 0:fo + h], psh[:pp, 0:fo + h])
                nc.sync.dma_start(hfull0[0][base:base + pp, :], hrow[:pp, :])
                if cc == 0:
                    adrow = bpool.tile([P, ADWE], tdt, tag="adrow")
                    nc.vector.tensor_copy(adrow[:pp, 0:h],
                                          psh[:pp, fo + h:fo + 2 * h])
                    nc.sync.dma_start(adloc[0][b * P:b * P + pp, :],
                                      adrow[:pp, :])

            prev = None
            for t in range(len(tiles) + 1):
                if t < len(tiles):
                    cc, b, pp = tiles[t]
                    ptr = front(cc, b, pp)
                if prev is not None:
                    back(*prev)
                prev = (cc, b, pp, ptr) if t < len(tiles) else None
            nc.sync.dma_start(adloc[0][cfg.ADPAD:cfg.ADPAD + 1, :], negrow_t[:])

        # ---------------- h table compute for layer li from x tile [pp, fi]
        def h_block(li, b, xp, pp):
            fi, fo, h = cfg.layers[li]
            kt = math.ceil(fi / P)
            psh = psum.tile([P, fo], f32, tag="psh", bufs=2)
            psa = psum.tile([P, 2 * h], f32, tag="psa")
            ptr = psum.tile([P, 4 * P], f32, tag="ptr", bufs=2)
            for k in range(kt):
                w = min(fi - k * P, P)
                nc.tensor.transpose(ptr[:w, k * P:k * P + pp],
                                    xp[:pp, k * P:k * P + w],
                                    ident_t[:pp, :pp])
            xts = xpool.tile([P, 4 * P], tdt, tag="xts")
            nc.vector.tensor_copy(xts[:, 0:kt * P], ptr[:, 0:kt * P])
            for k in range(kt):
                w = min(fi - k * P, P)
                nc.tensor.matmul(psh[:pp, :], lhsT=xts[:w, k * P:k * P + pp],
                                 rhs=w_t[li][:w, k, 0:fo],
                                 start=(k == 0), stop=(k == kt - 1))
                nc.tensor.matmul(psa[:pp, :], lhsT=xts[:w, k * P:k * P + pp],
                                 rhs=w_t[li][:w, k, fo:fo + 2 * h],
                                 start=(k == 0), stop=(k == kt - 1))
            kch = int(cfg.blk2chunk[b])
            rs = int(cfg.ag_bounds[kch])
            hrow = bpool.tile([P, cfg.RS[li]], tdt, tag="hrow")
            adrow = bpool.tile([P, ADWE], tdt, tag="adrow")
            nc.vector.tensor_tensor(hrow[:pp, 0:fo], psh[:pp, :],
                                    cor_t[li][:pp, 0:fo], sub)
            nc.vector.tensor_tensor(hrow[:pp, fo:fo + h], psa[:pp, 0:h],
                                    cor_t[li][:pp, fo:fo + h], sub)
            nc.vector.tensor_tensor(adrow[:pp, 0:h], psa[:pp, h:2 * h],
                                    cor_t[li][:pp, fo + h:fo + 2 * h], sub)
            r0 = b * P - rs
            nc.sync.dma_start(hlock[li][kch][r0:r0 + pp, :], hrow[:pp, :])
            nc.sync.dma_start(adloc[li][b * P:b * P + pp, :], adrow[:pp, :])

        def allgather_chunk(li, kch):
            if "noag" in ablate:
                return
            nc.gpsimd.collective_compute(
                "AllGather", mybir.AluOpType.bypass,
                replica_groups=[list(range(NCORES))],
                ins=[hlock[li][kch][:, :]],
                outs=[hfull[li][kch][:, :]])

        # ---------------- aggregation for layer li, block b (2-stage pipeline)
        # front: gathers + DVE prep (alpha, leaky, exp, Y*=e, one-hot build).
        # back:  PE scatter matmuls + softmax finish + ELU -> x' tile [pp, fo]
        # For AllGathered layers (li>=1) the block is processed in TWO phases:
        # phase A touches only lo-half sources (available early) and spills
        # partial sums to DRAM; phase B adds the hi-half (whose AllGather is
        # hidden under phase A) and finishes.  half: None=all, 0=lo, 1=hi.
        def agg_front(li, b, pp, half=None):
            fi, fo, h = cfg.layers[li]
            ch = fo // h
            R = cfg.RS[li]
            s = 0 if li == 0 else 1
            e = eps[s]
            if li == 0:
                tlo = hfull0[0][0:cfg.split, :]
                thi = hfull0[0][cfg.split:cfg.N, :]
            else:
                tlo = hfull[li][0][:, :]
                thi = hfull[li][1][:, :]
            nlo, nhi = int(e["nch"][b, 0]), int(e["nch"][b, 1])
            cb = int(e["chunk_base"][b])
            if half == 0:
                c0, C = 0, nlo
            elif half == 1:
                c0, C = nlo, nhi
            else:
                c0, C = 0, nlo + nhi
            if C == 0:
                return None
            sbase = (cb + c0) * P

            yt = ypool.tile([P, C, R], tdt, tag="yt")
            adg = apool.tile([P, C, ADWE], tdt, tag="adg")
            q0 = (b * 3) % 4
            if "nodma" in ablate:
                nc.vector.memset(yt[:, :, fo:fo + h], 0.25)
                nc.vector.memset(adg[:, :, 0:h], 0.0)
            else:
                glo = min(C, nlo - c0) if c0 < nlo else 0
                if glo:
                    nc.gpsimd.dma_gather(
                        yt[:, 0:glo, :], tlo,
                        gidx_t[s][:, sbase // 16:(sbase + glo * P) // 16],
                        glo * P, glo * P, R, single_packet=(glo * P <= 1024),
                        queue_num=q0)
                if C - glo:
                    ghi = C - glo
                    hb = sbase + glo * P
                    nc.gpsimd.dma_gather(
                        yt[:, glo:C, :], thi,
                        gidx_t[s][:, hb // 16:(hb + ghi * P) // 16],
                        ghi * P, ghi * P, R, single_packet=(ghi * P <= 1024),
                        queue_num=(q0 + 1) % 4)
                if "noad" in ablate:
                    nc.vector.memset(adg[:, :, 0:h], 0.0)
                else:
                    nc.gpsimd.dma_gather(
                        adg[:], adloc[li][:, :],
                        adix_t[s][:, sbase // 16:(sbase + C * P) // 16],
                        C * P, C * P, ADWE, single_packet=(C * P <= 1024),
                        queue_num=(q0 + 2) % 4)

            if "gonly" in ablate:
                acc = spool.tile([P, 4], f32, tag="at")
                nc.vector.tensor_tensor(acc[:], yt[:, 0, 0:4], adg[:, 0, 0:4], add)
                return (yt, None, C, cb)
            at = spool.tile([P, C * h], tdt, tag="at")
            nc.vector.tensor_tensor(
                at[:].rearrange("p (c h) -> p c h", h=h),
                yt[:, :, fo:fo + h], adg[:, :, 0:h], add)
            # leaky_relu: max(x, neg*x)  (DVE; Lrelu on ACT would thrash the
            # activation function table against Exp)
            at2 = spool.tile([P, C * h], tdt, tag="at2")
            nc.vector.tensor_scalar_mul(at2[:], at[:], cfg.neg)
            nc.vector.tensor_tensor(at[:], at[:], at2[:], vmax)
            # e = exp(alpha), written into the row's alpha_s columns
            nc.scalar.activation(yt[:, :, fo:fo + h],
                                 at[:].rearrange("p (c h) -> p c h", h=h), EXP)
            # Y *= e  (in place, per head; features head-interleaved so the
            # broadcast is on a middle dim and the last dim stays packed)
            nc.vector.tensor_tensor(
                yt[:, :, 0:fo].rearrange("p c (ch h) -> p c ch h", h=h),
                yt[:, :, 0:fo].rearrange("p c (ch h) -> p c ch h", h=h),
                yt[:, :, fo:fo + h].unsqueeze(2).to_broadcast([P, C, ch, h]),
                mult)
            # one-hot selection for this slot range
            st = stpool.tile([P, C, P], tdt, tag="st")
            nc.vector.tensor_tensor(
                st[:],
                dloc_t[s][:, cb + c0:cb + c0 + C].unsqueeze(2)
                        .to_broadcast([P, C, P]),
                iota_t[:].unsqueeze(1).to_broadcast([P, C, P]),
                is_eq)
            return (yt, st, C, cb)

        def _psy_matmuls(li, fr, folded, mw):
            fi, fo, h = cfg.layers[li]
            yt, st, C, cb = fr
            psy = psum.tile([P, mw], f32, tag="psy", bufs=2)
            if folded:
                pse = None
            else:
                pse = psum.tile([P, h], f32, tag="pse")
            for c in range(C):
                nc.tensor.matmul(psy[:, :], lhsT=st[:, c, :],
                                 rhs=yt[:, c, 0:mw],
                                 start=(c == 0), stop=(c == C - 1))
                if not folded:
                    nc.tensor.matmul(pse[:, :], lhsT=st[:, c, :],
                                     rhs=yt[:, c, fo:fo + h],
                                     start=(c == 0), stop=(c == C - 1))
            return psy, pse

        def agg_spill(li, b, pp, fr):
            """Phase A back: lo-half partial sums -> spill DRAM row block."""
            fi, fo, h = cfg.layers[li]
            folded = (fo + h) * 4 <= 2048
            mw = fo + h if folded else fo
            if fr is None or fr[1] is None:
                return
            psy, pse = _psy_matmuls(li, fr, folded, mw)
            spl = bpool.tile([P, fo + h], tdt, tag="spl")
            nc.vector.tensor_copy(spl[:pp, 0:mw], psy[:pp, 0:mw])
            if not folded:
                nc.vector.tensor_copy(spl[:pp, fo:fo + h], pse[:pp, :])
            nc.sync.dma_start(spillT[b * P:b * P + pp, 0:fo + h], spl[:pp, :])

        def agg_back(li, b, pp, fr, merge=False):
            fi, fo, h = cfg.layers[li]
            ch = fo // h
            folded = (fo + h) * 4 <= 2048   # denom rides in psy (PSUM bank)
            if fr is not None and fr[1] is None:  # gonly
                xp = bpool.tile([P, fo], f32, tag="xp")
                nc.vector.memset(xp[:pp, :], 0.5)
                return xp
            mw = fo + h if folded else fo
            spl = None
            if merge:
                spl = bpool.tile([P, fo + h], tdt, tag="spl2")
                nc.sync.dma_start(spl[:pp, :],
                                  spillT[b * P:b * P + pp, 0:fo + h])
            if fr is not None:
                psy, pse = _psy_matmuls(li, fr, folded, mw)
                psyv = psy[:pp, 0:fo]
                dsrc = psy[:pp, fo:fo + h] if folded else pse[:pp, :]
                if merge:
                    ysum = bpool.tile([P, fo + h], f32, tag="ysum")
                    nc.vector.tensor_tensor(ysum[:pp, 0:mw], psy[:pp, 0:mw],
                                            spl[:pp, 0:mw], add)
                    if not folded:
                        nc.vector.tensor_tensor(ysum[:pp, fo:fo + h],
                                                pse[:pp, :],
                                                spl[:pp, fo:fo + h], add)
                    psyv = ysum[:pp, 0:fo]
                    dsrc = ysum[:pp, fo:fo + h]
            else:
                assert merge
                psyv = spl[:pp, 0:fo]
                dsrc = spl[:pp, fo:fo + h]
            # finish: x' = relu(y+b) + min(exp(y+b), 1)   (true x = x' - 1)
            den = spool.tile([P, h], f32, tag="den")
            nc.vector.tensor_scalar_add(den[:pp, :], dsrc, 1e-16)
            rec = spool.tile([P, h], f32, tag="rec")
            nc.vector.reciprocal(rec[:pp, :], den[:pp, :])
            ysb = bpool.tile([P, fo], f32, tag="ysb")
            nc.vector.tensor_tensor(
                ysb[:pp, :].rearrange("p (ch h) -> p ch h", h=h),
                psyv.rearrange("p (ch h) -> p ch h", h=h),
                rec[:pp, :].unsqueeze(1).to_broadcast([pp, ch, h]), mult)
            if li in bias_t:
                nc.vector.tensor_tensor(ysb[:pp, :], ysb[:pp, :],
                                        bias_t[li][:pp, :], add)
            ex = bpool.tile([P, fo], f32, tag="ex")
            nc.scalar.activation(ex[:pp, :], ysb[:pp, :], EXP)
            nc.vector.tensor_scalar_min(ex[:pp, :], ex[:pp, :], 1.0)
            xp = bpool.tile([P, fo], f32, tag="xp")
            nc.vector.tensor_scalar_max(xp[:pp, :], ysb[:pp, :], 0.0)
            nc.vector.tensor_tensor(xp[:pp, :], xp[:pp, :], ex[:pp, :], add)
            return xp

        def mlp_block(b, xp, pp):
            # xp = x5' = x5+1 [pp, HS]; bm1 already corrected
            ptr = psum.tile([P, P], f32, tag="ptr", bufs=2)
            nc.tensor.transpose(ptr[:HS, :pp], xp[:pp, 0:HS], ident_t[:pp, :pp])
            xts = xpool.tile([P, P], tdt, tag="xts")
            nc.vector.tensor_copy(xts[:HS, :pp], ptr[:HS, :pp])
            ps1 = psum.tile([P, 4 * HS], f32, tag="psh", bufs=2)
            nc.tensor.matmul(ps1[:pp, :], lhsT=xts[:HS, :pp], rhs=wm1_t[:, :],
                             start=True, stop=True)
            r1 = bpool.tile([P, 4 * HS], f32, tag="r1")
            nc.vector.tensor_tensor(r1[:pp, :], ps1[:pp, :], bm1_t[:pp, :], add)
            nc.scalar.activation(r1[:pp, :], r1[:pp, :], RELU)
            ps2 = psum.tile([P, 4 * HS], f32, tag="psy", bufs=2)
            for k in range(2):
                ptr2 = psum.tile([P, P], f32, tag="ptr", bufs=2)
                nc.tensor.transpose(ptr2[:, :pp], r1[:pp, k * P:(k + 1) * P],
                                    ident_t[:pp, :pp])
                xts2 = xpool.tile([P, P], tdt, tag="xts")
                nc.vector.tensor_copy(xts2[:, :pp], ptr2[:, :pp])
                nc.tensor.matmul(ps2[:pp, :], lhsT=xts2[:, :pp],
                                 rhs=wm2_t[:, k, :], start=(k == 0), stop=(k == 1))
            r2 = bpool.tile([P, 4 * HS], f32, tag="r2")
            if bm2_t is not None:
                nc.vector.tensor_tensor(r2[:pp, :], ps2[:pp, :], bm2_t[:pp, :], add)
                nc.scalar.activation(r2[:pp, :], r2[:pp, :], RELU)
            else:
                nc.scalar.activation(r2[:pp, :], ps2[:pp, :], RELU)
            ps3 = psum.tile([P, 1], f32, tag="psa")
            for k in range(2):
                ptr3 = psum.tile([P, P], f32, tag="ptr", bufs=2)
                nc.tensor.transpose(ptr3[:, :pp], r2[:pp, k * P:(k + 1) * P],
                                    ident_t[:pp, :pp])
                xts3 = xpool.tile([P, P], tdt, tag="xts")
                nc.vector.tensor_copy(xts3[:, :pp], ptr3[:, :pp])
                nc.tensor.matmul(ps3[:pp, :], lhsT=xts3[:, :pp],
                                 rhs=wm3_t[:, k, :], start=(k == 0), stop=(k == 1))
            osb = bpool.tile([P, 1], f32, tag="osb")
            nc.scalar.activation(osb[:pp, :], ps3[:pp, :], SIGM, bias=bm3_t[:pp, :])
            nc.sync.dma_start(out_t[b * P:b * P + pp, :], osb[:pp, :])

        # ------------------------------------------------ program body
        for _rep in range(repeats):
         hfull0[0] = dram.tile([cfg.N, cfg.RS[0]], tdt, name=f"hfull0_{_rep}",
                               tag=f"hfull0_{_rep}")
         for li in range(1, 4):
            hfull[li] = [
                dram.tile([NCORES * int(cfg.ag_bounds[k + 1] - cfg.ag_bounds[k]),
                           cfg.RS[li]], tdt, addr_space="Shared",
                          name=f"hfull{li}_{k}_{_rep}",
                          tag=f"hfull{li}_{k}_{_rep}")
                for k in range(nchunk)]
            hlock[li] = [
                dram.tile([int(cfg.ag_bounds[k + 1] - cfg.ag_bounds[k]),
                           cfg.RS[li]], tdt,
                          name=f"hlock{li}_{k}_{_rep}", tag=f"hlock{li}_{k}_{_rep}")
                for k in range(nchunk)]
         sweep0()
         for li in range(4):
            def back_full(li, b, pp, fr, merge=False):
                xp = agg_back(li, b, pp, fr, merge=merge)
                if li == stop_after:
                    osb = bpool.tile([P, 1], f32, tag="osb")
                    nc.scalar.activation(osb[:pp, :], xp[:pp, 0:1],
                                         mybir.ActivationFunctionType.Copy)
                    nc.sync.dma_start(out_t[b * P:b * P + pp, :], osb[:pp, :])
                elif li < 3:
                    h_block(li + 1, b, xp, pp)
                    if b in cfg.chunk_last_blk:
                        allgather_chunk(li + 1, cfg.chunk_last_blk.index(b))
                    if b == NBLK - 1:
                        nc.sync.dma_start(
                            adloc[li + 1][cfg.ADPAD:cfg.ADPAD + 1, :],
                            negrow_t[:])
                else:
                    mlp_block(b, xp, pp)

            if li == 0 or "spill" not in ablate:
                prev = None
                for b, pp in blocks():
                    fr = agg_front(li, b, pp)
                    if prev is not None:
                        back_full(li, *prev)
                    prev = (b, pp, fr)
                back_full(li, *prev)
            else:
                e1 = eps[1]
                # phase A: lo-half partial sums while the hi AllGather flies
                prev = None
                for b, pp in blocks():
                    fr = agg_front(li, b, pp, half=0)
                    if prev is not None:
                        agg_spill(li, *prev)
                    prev = (b, pp, fr)
                agg_spill(li, *prev)
                # phase B: hi halves, merge, finish
                prev = None
                for b, pp in blocks():
                    fr = agg_front(li, b, pp, half=1)
                    if prev is not None:
                        back_full(li, *prev[:3],
                                  merge=bool(e1["nch"][prev[0], 0]))
                    prev = (b, pp, fr)
                back_full(li, *prev[:3], merge=bool(e1["nch"][prev[0], 0]))
            if li == stop_after:
                break

    nc.compile()
    return nc


# ------------------------------------------------------------------ execution

def make_in_maps(cfg, ep, w, inputs):
    import ml_dtypes
    ndt = ml_dtypes.bfloat16 if cfg.bf16 else np.float32
    x = np.asarray(inputs["x"], np.float32)
    iota = np.tile(np.arange(P, dtype=np.float32)[None, :], (P, 1)).astype(ndt)
    ident = np.eye(P, dtype=np.float32)
    negrow = np.full((1, cfg.ADWE), -1e30, ndt)
    in_maps = []
    for c in range(NCORES):
        m = dict(
            x=np.ascontiguousarray(np.roll(x, -c * cfg.NPC, axis=0)),
            iota=iota, ident=ident, negrow=negrow,
            wm1=w["wm1"], wm2=w["wm2"], wm3=w["wm3"],
            bm1=w["bm1"], bm3=w["bm3"])
        for s, e in ((0, ep["ep0"]), (1, ep["ep1"])):
            m[f"gidx{s}"] = e["gidx"][c]
            m[f"adix{s}"] = e["adix"][c]
            m[f"dloc{s}"] = e["dloc"][c]
        if w["bm2_nz"]:
            m["bm2"] = w["bm2"]
        for li in range(4):
            m[f"w{li}"] = w[f"w{li}"]
            if li > 0:
                m[f"cor{li}"] = w[f"cor{li}"]
            if w[f"bias{li}_nz"]:
                m[f"bias{li}"] = w[f"bias{li}"]
        in_maps.append(m)
    return in_maps


_CACHE = {}


def _get_compiled(cfg, inputs):
    ep = prep_edges(cfg, inputs["edge_index"])
    w = prep_weights(cfg, inputs)
    key = (ep["ep0"]["gidx"].tobytes(), ep["ep1"]["gidx"].tobytes(),
           w["bm2_nz"], tuple(w[f"bias{li}_nz"] for li in range(4)))
    ck = hash(key)
    if ck not in _CACHE:
        _CACHE[ck] = build_program(cfg, ep, w)
    return _CACHE[ck], ep, w


def kernel(**inputs):
    from concourse import bass_utils
    cfg = CFG
    nc, ep, w = _get_compiled(cfg, inputs)
    in_maps = make_in_maps(cfg, ep, w, inputs)
    res = bass_utils.run_bass_kernel_spmd(nc, in_maps, core_ids=list(range(NCORES)))
    out = np.concatenate([res.results[c]["out"] for c in range(NCORES)], axis=0)
    return out.astype(np.float32)

